# revision 20
# baseline (speedup 1.0000x reference)
"""MoE expert-routing kernel for Trainium2 (8 NeuronCores, expert-parallel).

Problem: out[t] = x[t] @ weight[index[t]] + bias[index[t]]
  x: (32768, 512) f32, index: (32768,) int, weight: (8, 512, 512) f32,
  bias: (8, 512) f32.

Strategy (expert-parallel, host-side dispatch):
  Core e owns expert e. The host gathers the tokens routed to expert e
  into a fixed-capacity, transposed buffer xt_e[512, CAP] (padded with
  zeros), and core e computes y_e = x_e @ W_e + b_e as a single dense
  GEMM. Results are scattered back to token order on the host. Tokens
  beyond CAP (doesn't happen for the benchmark distribution: observed
  per-expert maxima 4205/4166 vs CAP 4224) fall back to a host matmul,
  so the kernel stays correct for any index distribution.

Device kernel (per core): y = x_e @ W_e + b_e over CAP=4224 tokens
  - Host packs x_e pre-transposed AND slab-contiguous (single
    contiguous run per partition per slab DMA; no strided descriptors).
  - Startup is DGE-latency-bound (~0.6us issue + ~2.5-3.5us to the
    completion semaphore, bandwidth-shared across rings), so the first
    accumulation group's inputs ride in small per-gate DMAs spread
    over both HWDGE rings in consumption order: [xs0|w0] on the SP
    ring; w1, [w2|w3], bias(fp16) on the ACT ring.
  - PE p-state warmup: the tensor engine clocks ~1.2GHz until it has
    been busy ~3us, then 2.4GHz. Throwaway matmuls on a memset scratch
    tile (single accumulation groups - no inter-matmul semaphores)
    fill the DMA-latency window; the real GEMM then runs fully ramped
    at ~216ns per [128x128]@[128x512] fp16 matmul - the PE roofline.
  - Token slabs (128/128/256 ramp-in, 512 steady, 384/128/128 tail)
    stream through SBUF; per 128-token tile, 4 accumulating matmuls
    into one PSUM bank; DVE adds the (pre-replicated) bias while
    moving PSUM->SBUF. Outputs go out on the ACT HWDGE ring except the
    last two slabs, which use the (by then idle) SP ring so the final
    transfer is not queued behind earlier output descriptors.
  - Operands and output are fp16 (values are O(1); PSUM accumulation
    stays fp32): absmax 2.7e-3 on scale-5.5 outputs (4.9e-4 relative).
"""

import os

import numpy as np

N_EXPERTS = 8
D_IN = 512
D_OUT = 512
N_TOKENS = 32768
CAP = 4224  # per-expert token capacity: 33*128; host fallback covers overflow
TOK_SLAB = 512
KC = D_IN // 128  # 4 contraction chunks


def _slab_schedule():
    head_sizes = [128, 128, 256]
    tail_sizes = [128, 128]
    sizes = list(head_sizes)
    remaining = CAP - sum(head_sizes) - sum(tail_sizes)
    while remaining > 0:
        sizes.append(min(TOK_SLAB, remaining))
        remaining -= sizes[-1]
    sizes.extend(tail_sizes)
    slabs = []
    t0 = 0
    for ts in sizes:
        slabs.append((t0, ts))
        t0 += ts
    assert t0 == CAP
    return slabs


SLABS = _slab_schedule()
Y_FREE = (CAP // 128) * D_OUT  # packed output free size per partition
HEAD_TOK = SLABS[0][1]  # tokens in slab 0 (rides in the head pack)
HEAD_FREE = KC * HEAD_TOK + KC * D_OUT  # [xs0 | w0 | w1 | w2 | w3]

# mode -> (x dtype, w dtype, y dtype); x and w must match (packed DMAs).
MM_DTYPE = os.environ.get("KERNEL_MM_DTYPE", "float16_o16")
_DT_MAP = {
    "float32": ("float32", "float32", "float32"),
    "float32r": ("float32r", "float32r", "float32"),
    "float32r_o16": ("float32r", "float32r", "float16"),
    "bfloat16": ("bfloat16", "bfloat16", "float32"),
    "float16": ("float16", "float16", "float32"),
    "float16_o16": ("float16", "float16", "float16"),
}

_cache = {}


def _build(mm_dtype_name):
    import concourse.bacc as bacc
    import concourse.mybir as mybir
    import concourse.tile as tile

    x_dt_name, w_dt_name, y_dt_name = _DT_MAP[mm_dtype_name]
    assert x_dt_name == w_dt_name
    dt_x = getattr(mybir.dt, x_dt_name)
    dt_y = getattr(mybir.dt, y_dt_name)
    f32 = mybir.dt.float32

    nc = bacc.Bacc("TRN2", target_bir_lowering=False, debug=False, num_devices=N_EXPERTS)
    # Slab-contiguous packed layouts: one contiguous run per partition
    # per slab DMA. head = [xs_slab0 | w chunk0]; xt's slab-0 region is
    # unused (kept so the host packer stays uniform).
    xt = nc.dram_tensor("xt", (128, KC * CAP), dt_x, kind="ExternalInput").ap()
    head = nc.dram_tensor("head", (128, HEAD_FREE), dt_x, kind="ExternalInput").ap()
    b = nc.dram_tensor("b", (128, D_OUT), dt_x, kind="ExternalInput").ap()
    y = nc.dram_tensor("y", (128, Y_FREE), dt_y, kind="ExternalOutput").ap()

    with tile.TileContext(nc) as tc:
        with (
            tc.tile_pool(name="wpool", bufs=1) as wpool,
            tc.tile_pool(name="bias", bufs=1) as bias_pool,
            tc.tile_pool(name="warm", bufs=1) as warm_pool,
            tc.tile_pool(name="xslab", bufs=4) as xpool,
            tc.tile_pool(name="ystage", bufs=8) as ypool,
            tc.tile_pool(name="psum", bufs=6, space="PSUM") as pspool,
            tc.tile_pool(name="wpsum", bufs=2, space="PSUM") as warm_ps_pool,
        ):
            slabs = SLABS

            # PE p-state warmup (see module docstring). The scratch memset
            # runs on GpSimd (free right after the framework preamble);
            # coarse 512-col warmups first, then 128-col ones so the
            # handoff to the first real matmul quantizes finely.
            scratch = warm_pool.tile([128, D_OUT], dt_x, tag="scr")
            nc.gpsimd.memset(scratch[:], 0.0)
            wps_a = warm_ps_pool.tile([128, D_OUT], f32, tag="wacc")
            for i in range(7):
                nc.tensor.matmul(
                    wps_a[:], scratch[:, 0:128], scratch[:],
                    start=(i == 0), stop=(i == 6),
                )
            wps_b = warm_ps_pool.tile([128, D_OUT], f32, tag="wacc")
            for i in range(16):
                nc.tensor.matmul(
                    wps_b[0:16, 0:16], scratch[:, 0:16], scratch[:, 0:16],
                    start=(i == 0), stop=(i == 15),
                )
            for i in range(16):
                nc.tensor.matmul(
                    wps_b[:, 16:144], scratch[:, 0:128], scratch[:, 0:128],
                    start=(i == 0), stop=(i == 15),
                )

            # Startup DMAs: one packed transfer on the SP ring carries
            # everything the first groups need; bias rides the ACT ring.
            head_sb = wpool.tile([128, HEAD_FREE], dt_x, tag="head", name="head_sb")
            b_rep = bias_pool.tile([128, D_OUT], dt_x, tag="brep")
            nc.sync.dma_start(head_sb[:], head[:])
            nc.scalar.dma_start(b_rep[:], b[:])

            xs0_off = KC * HEAD_TOK
            w_aps = [
                head_sb[:, xs0_off + k * D_OUT : xs0_off + (k + 1) * D_OUT]
                for k in range(KC)
            ]

            def load_x(slab_i):
                t0, ts = slabs[slab_i]
                xs = xpool.tile([128, KC * ts], dt_x, tag="xs")
                nc.sync.dma_start(xs[:], xt[:, KC * t0 : KC * (t0 + ts)])
                return xs

            xs_pending = load_x(1)

            n_slabs = len(slabs)
            for i, (t0, ts) in enumerate(slabs):
                nt = ts // 128
                if i == 0:
                    xs = head_sb[:, 0:xs0_off]
                else:
                    xs = xs_pending[:]
                    if i + 1 < n_slabs:
                        xs_pending = load_x(i + 1)
                ys = ypool.tile([128, nt * D_OUT], dt_y, tag="ys")
                last = i == n_slabs - 1
                o0 = (t0 // 128) * D_OUT
                for a in range(nt):
                    ps = pspool.tile([128, D_OUT], f32, tag="acc")
                    for k in range(KC):
                        nc.tensor.matmul(
                            ps[:],
                            xs[:, k * ts + a * 128 : k * ts + (a + 1) * 128],
                            w_aps[k],
                            start=(k == 0),
                            stop=(k == KC - 1),
                        )
                    if last:
                        # Final tile: split the bias-add and the store into
                        # halves on separate rings so the last transfer
                        # overlaps the second half's add.
                        h = D_OUT // 2
                        nc.vector.tensor_add(
                            ys[:, 0:h], ps[:, 0:h], b_rep[:, 0:h]
                        )
                        nc.scalar.dma_start(y[:, o0 : o0 + h], ys[:, 0:h])
                        nc.vector.tensor_add(
                            ys[:, h : D_OUT], ps[:, h:D_OUT], b_rep[:, h:D_OUT]
                        )
                        nc.sync.dma_start(
                            y[:, o0 + h : o0 + D_OUT], ys[:, h : D_OUT]
                        )
                    else:
                        nc.vector.tensor_add(
                            ys[:, a * D_OUT : (a + 1) * D_OUT], ps[:], b_rep[:]
                        )
                if not last:
                    # Outputs ride the ACT HWDGE ring; the second-to-last
                    # slab uses the idle SP ring.
                    eng = nc.sync if i == n_slabs - 2 else nc.scalar
                    eng.dma_start(y[:, o0 : o0 + nt * D_OUT], ys[:])
    nc.compile()
    return nc


def _get_nc(mm_dtype_name):
    if mm_dtype_name not in _cache:
        _cache[mm_dtype_name] = _build(mm_dtype_name)
    return _cache[mm_dtype_name]


def kernel(x, index, weight, bias, _trace=False):
    from concourse.bass_utils import run_bass_kernel_spmd

    x = np.ascontiguousarray(np.asarray(x, dtype=np.float32))
    weight = np.ascontiguousarray(np.asarray(weight, dtype=np.float32))
    bias = np.ascontiguousarray(np.asarray(bias, dtype=np.float32))
    idx = np.asarray(index).astype(np.int64, copy=False)

    ids = [np.nonzero(idx == e)[0] for e in range(N_EXPERTS)]

    in_maps = []
    for e in range(N_EXPERTS):
        n_e = min(len(ids[e]), CAP)
        x_e = np.zeros((CAP, D_IN), dtype=np.float32)
        x_e[:n_e] = x[ids[e][:n_e]]
        # Pack slab-major: xt_e[p, KC*t0 + kc*ts + t] = x_e[t0+t, kc*128+p]
        xt_e = np.empty((128, KC * CAP), dtype=np.float32)
        for t0, ts in SLABS:
            blk = x_e[t0 : t0 + ts].reshape(ts, KC, 128)  # [t, kc, p]
            xt_e[:, KC * t0 : KC * (t0 + ts)] = (
                blk.transpose(2, 1, 0).reshape(128, KC * ts)
            )
        w_e = weight[e]
        head_e = np.concatenate(
            [xt_e[:, 0 : KC * HEAD_TOK]]
            + [w_e[k * 128 : (k + 1) * 128, :] for k in range(KC)],
            axis=1,
        )
        in_maps.append(
            {
                "xt": xt_e,
                "head": np.ascontiguousarray(head_e),
                "b": np.ascontiguousarray(
                    np.broadcast_to(bias[e], (128, D_OUT))
                ),
            }
        )

    x_dt_name, _, _ = _DT_MAP[MM_DTYPE]
    cast = {"bfloat16": None, "float16": np.float16, "float32": np.float32,
            "float32r": np.float32}
    ct = cast[x_dt_name]
    if ct is None:
        import ml_dtypes

        ct = ml_dtypes.bfloat16
    in_maps = [
        {
            **m,
            "xt": m["xt"].astype(ct),
            "head": m["head"].astype(ct),
            "b": m["b"].astype(ct),
        }
        for m in in_maps
    ]

    nc = _get_nc(MM_DTYPE)
    res = run_bass_kernel_spmd(
        nc, in_maps, core_ids=list(range(N_EXPERTS)), trace=_trace
    )

    out = np.empty((x.shape[0], D_OUT), dtype=np.float32)
    for e in range(N_EXPERTS):
        n_e = min(len(ids[e]), CAP)
        # Unpack [p, a_global, o] -> token-major [a_global*128+p, o]
        y_pm = res.results[e]["y"].reshape(128, CAP // 128, D_OUT)
        y_e = y_pm.transpose(1, 0, 2).reshape(CAP, D_OUT)
        out[ids[e][:n_e]] = y_e[:n_e].astype(np.float32)
        if len(ids[e]) > CAP:  # capacity overflow: host fallback (correctness net)
            over = ids[e][CAP:]
            out[over] = x[over] @ weight[e] + bias[e]

    if _trace:
        return out, res
    return out
